# revision 25
# baseline (speedup 1.0000x reference)
"""Mode-adaptive linear (MoE soft routing) Trainium2 kernel, v3.

out[b, o] = sum_c weights[b, c] * (inputs[b, :] @ w[c])[o] + (weights @ bias)[b, o]

Data-parallel shard of the batch across 8 NeuronCores (1024 rows each);
w/bias replicated.  Routing weights are folded into the transposed input
tiles (xs_c = wt_c * X^T, bf16), so each 128-row batch tile accumulates its
whole contraction in PSUM.

Blocked 2x2: tile-halves (0-3, 4-7) x expert-groups (0-3, 4-7), processed
(H0,G0) (H1,G0) (H0,G1) (H1,G1); within a block experts are OUTER and tiles
inner, so the 8MB expert DMA stream is consumed at ~1.2 GB per 3.4us pass
and the stream starts after only ~1.3MB of front-loaded DMA.  A block keeps
4 PSUM accumulators open; the other 4 banks cycle between PE X^T transposes
(bf16) and the next block.  G0 blocks open with the bias matmul
(lhsT = padded wt^T) and close into an SBUF accumulator; G1 blocks re-open
the bank and the epilogue adds both on DVE before the output DMA.

Engine split: PE = 264 stream matmuls + 32 bf16 transposes + 8 tiny wt
transposes; DVE = casts + routing muls + final adds; ScalarE = psum->SBUF
copies; GpSimd = routing-weight partition_broadcast; DMA on both HWDGE
queues (SP: x/wt/b/out, Activation: w).
"""

import json
import types

import numpy as np

import concourse.bass as bass
import concourse.mybir as mybir
import concourse.tile as tile
from concourse.bass import ts
from concourse.bass_utils import run_bass_kernel_spmd

N_CORES = 8
B, D_IN, D_OUT, N_CTRL = 8192, 512, 512, 8
B_SHARD = B // N_CORES          # 1024 rows per core
P = 128
N_TILES = B_SHARD // P          # 8 batch tiles per core
KS = D_IN // P                  # 4 K-chunks of 128
HALF = N_TILES // 2             # 4 tiles per half
GRP = N_CTRL // 2               # 4 experts per group
F32 = mybir.dt.float32
BF16 = mybir.dt.bfloat16

N_WARM = 4                      # dummy PE matmuls to release the HAM clock gate


def _body(nc: bass.Bass, tc: tile.TileContext, x_d, wt_d, w_d, b_d, o_d,
          identity_bf, e_pad, identity_d=None, e_pad_d=None):
    with (
        tc.tile_pool(name="const", bufs=1) as const_pool,
        tc.tile_pool(name="xbf", bufs=3) as xbfpool,
        tc.tile_pool(name="xt", bufs=N_TILES) as xtpool,
        tc.tile_pool(name="wstage", bufs=4) as wstage,
        tc.tile_pool(name="xs", bufs=4) as xspool,
        tc.tile_pool(name="o", bufs=4) as opool,
        tc.tile_pool(name="ps", bufs=8, space="PSUM") as psum,
    ):
        # --- PE warm-up on a zero tile (no DMA dependencies) ---
        warm = const_pool.tile([P, P], BF16)
        nc.vector.memset(warm, 0.0)
        warm_ps = psum.tile([P, P], F32, tag="ps", name="warm_ps")
        for _ in range(N_WARM):
            nc.tensor.matmul(warm_ps, lhsT=warm, rhs=warm, start=True, stop=True)

        # --- SP-queue DMA (generation cost ~625ns per DMA instruction, so
        # batch): wt, x0, x1-3, bias; x4-7 goes after the wrow staging DMA ---
        wt_nat = const_pool.tile([P, N_TILES, N_CTRL], F32)
        nc.sync.dma_start(wt_nat, wt_d.rearrange("(t p) c -> p t c", p=P))
        if identity_d is not None:
            nc.sync.dma_start(identity_bf, identity_d.ap())
        xg = [
            const_pool.tile([P, 1, D_IN], F32, name="xg0"),
            const_pool.tile([P, 3, D_IN], F32, name="xg1"),
            const_pool.tile([P, 4, D_IN], F32, name="xg2"),
        ]
        nc.sync.dma_start(xg[0], x_d[0:P, None, :])
        nc.sync.dma_start(
            xg[1], x_d[P:4 * P, :].rearrange("(u p) i -> p u i", p=P)
        )
        if e_pad_d is not None:
            nc.sync.dma_start(e_pad, e_pad_d.ap())

        def x_slice(t):
            if t == 0:
                return xg[0][:, 0, :]
            if t <= 3:
                return xg[1][:, t - 1, :]
            return xg[2][:, t - 4, :]

        # --- Activation-queue DMA: the 8MB expert stream.  Expert 0 in 4
        # chunk DMAs (fast start), experts 1-7 one DMA each.  Layout:
        # w_sb[p, c, k, o] = w[c, 128k+p, o] ---
        w_sb = const_pool.tile([P, N_CTRL, KS, D_OUT], BF16)
        w_f32s = {}

        def w_dma(c):
            if c == 0:
                for k in range(KS):
                    w_f32 = wstage.tile([P, D_OUT], F32, tag="w_f32c", bufs=4)
                    nc.scalar.dma_start(w_f32, w_d[c, ts(k, P), :])
                    w_f32s[(c, k)] = w_f32
            else:
                w_f32 = wstage.tile([P, KS, D_OUT], F32, tag="w_f32e", bufs=3)
                nc.scalar.dma_start(
                    w_f32, w_d[c].rearrange("(k p) o -> p k o", p=P)
                )
                w_f32s[c] = w_f32

        def w_cast(c, k=None):
            if c == 0:
                for kk in range(KS) if k is None else [k]:
                    if (0, kk) in w_f32s:
                        nc.vector.tensor_copy(w_sb[:, 0, kk], w_f32s.pop((0, kk)))
            elif c in w_f32s:
                nc.vector.tensor_copy(w_sb[:, c], w_f32s.pop(c))

        w_dma(0)
        b_f32 = const_pool.tile([N_CTRL, D_OUT], F32)
        nc.sync.dma_start(b_f32, b_d)

        xts = {}

        def x_prep(t):
            # DVE cast to bf16, then XBAR DMA transpose (no PE, no PSUM):
            # xt[p, k, b] = x[b, 128k+p]
            x_bf = xbfpool.tile([P, D_IN], BF16, tag="x_bf")
            nc.vector.tensor_copy(x_bf, x_slice(t))
            xt = xtpool.tile([P, KS, P], BF16, tag="xt")
            nc.sync.dma_start_transpose(xt, x_bf)
            xts[t] = xt

        # zero-padding memsets early (DVE, cheap)
        wt_pad = const_pool.tile([P, B_SHARD], BF16)
        nc.vector.memset(wt_pad, 0.0)
        b_pad = const_pool.tile([P, D_OUT], BF16)
        nc.vector.memset(b_pad, 0.0)
        nc.vector.tensor_copy(b_pad[0:N_CTRL, :], b_f32)

        # --- wt^T via PE transpose (bf16), zero-padded to 128 partitions:
        # wt_pad[c, 128t + b] = weights[128t + b, c] ---
        wt_nbf = const_pool.tile([P, N_TILES, N_CTRL], BF16)
        nc.vector.tensor_copy(wt_nbf, wt_nat)
        for h in range(2):
            wtt_ps = psum.tile([N_CTRL, HALF, P], BF16, tag="ps", name="wtt_ps")
            for i in range(HALF):
                t = h * HALF + i
                nc.tensor.transpose(wtt_ps[:, i, :], wt_nbf[:, t, :], identity_bf)
                nc.scalar.copy(wt_pad[0:N_CTRL, ts(t, P)], wtt_ps[:, i, :])

        nc.sync.dma_start(
            xg[2], x_d[4 * P:8 * P, :].rearrange("(u p) i -> p u i", p=P)
        )

        # --- routing-weight broadcast Wb[p, c, b] = weights[b, c] via PE:
        # matmul(lhsT=e_pad[:, c], rhs=wt_pad) replicates wt_pad row c to
        # all 128 partitions.  All 16 matmuls run before the accumulators
        # claim the PSUM banks; psum->wb copies alternate ScalarE/DVE. ---
        wb = const_pool.tile([P, N_CTRL, B_SHARD], BF16)
        for c in range(N_CTRL):
            for h in range(2):
                bc_ps = psum.tile([P, B_SHARD // 2], F32, tag="ps", name="bc_ps")
                nc.tensor.matmul(
                    bc_ps, lhsT=e_pad[:, c, :], rhs=wt_pad[:, ts(h, B_SHARD // 2)],
                    start=True, stop=True,
                )
                if (c + h) % 2 == 0:
                    nc.scalar.copy(wb[:, c, ts(h, B_SHARD // 2)], bc_ps)
                else:
                    nc.vector.tensor_copy(wb[:, c, ts(h, B_SHARD // 2)], bc_ps)

        def xs_mul(c, t):
            xs = xspool.tile([P, KS, P], BF16, tag="xs")
            nc.vector.tensor_mul(
                xs,
                xts[t],
                wb[:, c, None, ts(t, P)].to_broadcast([P, KS, P]),
            )
            return xs

        # --- X^T for all 8 tiles (PSUM banks are free until the
        # accumulators open) ---
        for t in range(N_TILES):
            x_prep(t)

        w_dma(1)

        # --- the stream: bias pass opens the 8 accumulators (cheap PE work
        # while the w stream ramps), then experts 0..7 over all 8 tiles ---
        accs = []
        for t in range(N_TILES):
            acc = psum.tile([P, D_OUT], F32, tag="ps", name="acc")
            nc.tensor.matmul(
                acc, lhsT=wt_pad[:, ts(t, P)], rhs=b_pad,
                start=True, stop=False,
            )
            accs.append(acc)

        w_cast(0)
        for c in range(N_CTRL):
            if c + 2 < N_CTRL:
                w_dma(c + 2)                     # trigger 2 passes ahead
            for t in range(N_TILES):
                xs = xs_mul(c, t)
                if t == 3 and c + 1 < N_CTRL:
                    w_cast(c + 1)                # ~1.1us DVE, mid-pass
                for k in range(KS):
                    nc.tensor.matmul(
                        accs[t],
                        lhsT=xs[:, k, :],
                        rhs=w_sb[:, c, k, :],
                        start=False,
                        stop=(c == N_CTRL - 1 and k == KS - 1),
                    )
                if c == N_CTRL - 1:
                    # epilogue: ScalarE copy then per-tile output DMA
                    o_sb = opool.tile([P, D_OUT], F32, tag="o_sb")
                    nc.scalar.copy(o_sb, accs[t])
                    nc.sync.dma_start(o_d[ts(t, P), :], o_sb)


def _split_multi_waits(bir: dict) -> dict:
    """The walrus build in this container supports at most ONE sync-wait per
    instruction ("Too many sync wait commands" at codegen otherwise).  Tile's
    scheduler freely attaches several.  Split: keep the last wait on the
    instruction and hoist the others onto standalone same-engine
    EventSemaphore instructions inserted immediately before it — identical
    semantics (the engine blocks at the same program point)."""
    ctr = 0
    for func in bir["functions"]:
        for bb in func["blocks"]:
            new_insts = []
            for inst in bb["instructions"]:
                si = inst.get("sync_info")
                waits = si.get("on_wait") if si else None
                if waits and len(waits) > 1:
                    for w in waits[:-1]:
                        ctr += 1
                        new_insts.append(
                            {
                                "debug": inst.get("debug", 0),
                                "engine": inst["engine"],
                                "ins": [],
                                "outs": [],
                                "name": f"{inst['name']}-wsplit{ctr}",
                                "opcode": "EventSemaphore",
                                "sync_info": {"on_update": [], "on_wait": [w]},
                            }
                        )
                    si["on_wait"] = [waits[-1]]
                new_insts.append(inst)
            bb["instructions"] = new_insts
    return bir


_ORIG_TO_JSON_BYTES = bass.Bass.to_json_bytes


def _patched_to_json_bytes(self) -> bytes:
    bir = json.loads(_ORIG_TO_JSON_BYTES(self))
    _split_multi_waits(bir)
    return json.dumps(bir).encode()


_NC_CACHE = {}


def _build(repeats: int = 1, loop: bool = False) -> bass.Bass:
    key = (repeats, loop)
    if key in _NC_CACHE:
        return _NC_CACHE[key]
    nc = bass.Bass(
        "TRN2",
        target_bir_lowering=False,
        debug=False,
        enable_asserts=False,
        num_devices=N_CORES,
    )
    x_d = nc.dram_tensor("x_in", [B_SHARD, D_IN], F32, kind="ExternalInput").ap()
    wt_d = nc.dram_tensor("wt_in", [B_SHARD, N_CTRL], F32, kind="ExternalInput").ap()
    w_d = nc.dram_tensor("w_in", [N_CTRL, D_IN, D_OUT], F32, kind="ExternalInput").ap()
    b_d = nc.dram_tensor("b_in", [N_CTRL, D_OUT], F32, kind="ExternalInput").ap()
    o_d = nc.dram_tensor("out", [B_SHARD, D_OUT], F32, kind="ExternalOutput").ap()
    with tile.TileContext(nc) as tc:
        with tc.tile_pool(name="global_const", bufs=1) as gconst:
            import ml_dtypes
            identity_d = nc.inline_tensor(
                np.eye(P, dtype=ml_dtypes.bfloat16), name="identity_const"
            )
            identity_bf = gconst.tile([P, P], BF16)
            e_np = np.zeros((P, N_CTRL, P), dtype=ml_dtypes.bfloat16)
            for c in range(N_CTRL):
                e_np[c, c, :] = 1.0
            e_pad_d = nc.inline_tensor(e_np, name="e_pad_const")
            e_pad = gconst.tile([P, N_CTRL, P], BF16)
            if loop:
                nc.sync.dma_start(identity_bf, identity_d.ap())
                nc.sync.dma_start(e_pad, e_pad_d.ap())
                with tc.For_i(0, repeats, 1):
                    _body(nc, tc, x_d, wt_d, w_d, b_d, o_d,
                          identity_bf, e_pad)
            else:
                for r in range(repeats):
                    _body(nc, tc, x_d, wt_d, w_d, b_d, o_d,
                          identity_bf, e_pad,
                          identity_d if r == 0 else None,
                          e_pad_d if r == 0 else None)
    nc.to_json_bytes = types.MethodType(_patched_to_json_bytes, nc)
    _NC_CACHE[key] = nc
    return nc


def make_in_maps(inputs_dict):
    inputs = np.ascontiguousarray(inputs_dict["inputs"], dtype=np.float32)
    weights = np.ascontiguousarray(inputs_dict["weights"], dtype=np.float32)
    w = np.ascontiguousarray(inputs_dict["w"], dtype=np.float32)
    b = np.ascontiguousarray(inputs_dict["b"], dtype=np.float32)
    in_maps = []
    for i in range(N_CORES):
        sl = slice(i * B_SHARD, (i + 1) * B_SHARD)
        in_maps.append(
            {
                "x_in": inputs[sl],
                "wt_in": weights[sl],
                "w_in": w,
                "b_in": b,
            }
        )
    return in_maps


def kernel(inputs, weights, w, b, _trace=False):
    nc = _build()
    in_maps = make_in_maps(
        {"inputs": inputs, "weights": weights, "w": w, "b": b}
    )
    res = run_bass_kernel_spmd(
        nc, in_maps, core_ids=list(range(N_CORES)), trace=_trace
    )
    out = np.concatenate([r["out"] for r in res.results], axis=0)
    if _trace:
        return out, res
    return out
